# revision 9
# baseline (speedup 1.0000x reference)
"""Trainium2 Bass kernel for nn_DistanceLoss (contrastive loss over cosine
similarity matrices).

Math restructure (vs the reference):
  loss = [ sum_i i*ld[i] - sum_{i>j} pos[i,j] ] / n_terms
where ld = logsumexp_k(neg[i,k]).  pos = (p1 @ p1.T)/T is symmetric with
diagonal 1/T, so the strict-lower-triangular sum collapses to
  ( ||sum_i p1_i||^2 / T - B/T ) / 2,
needing only the column-sum s of normalized batch1.  Only
neg = p1n @ p2n.T needs real compute.

Sharding: 2x4 grid.  Row-groups r=0,1 split batch1 rows (2048 each);
col-groups c=0..3 split batch2 rows (1024 each).  Core = r*4 + c computes a
[2048, 1024] block of neg and emits partial denominators
D[i] = sum_{k in slice} exp(neg[i,k]); the host sums the 4 partials per
row-group, takes log, and does the final tiny reduction in float64.

Host-side prep is layout/cast only: fp8e4 casts and a pre-transposed copy
of the batch1 strip (b1T) so the device does zero b1-side transposes.  All
normalization math stays on device:
  - ssq/rsqrt of both batches on device (DVE/GpSimd STT + ACT Ln/Exp)
  - batch2 rows are normalized (x10 = 1/TEMP) during the PE diag-transpose
  - batch1 rows are normalized by folding inv1[i] into the ACT Exp *scale
    vector* (per-partition AP) -- the main matmul consumes raw fp8 b1T.
Main matmul runs fp8 DoubleRow (2 c-chunks per pass).  A single manual
ACT table load (natural_log_exp_and_others serves Exp/Ln/Copy/Square)
avoids the per-switch 1.28us table reloads.
"""

import numpy as np
import ml_dtypes

B = 4096
C = 512
NCORES = 8
MR = 2                    # row groups (batch1 split)
MC = 4                    # col groups (batch2 split)
ROWS = B // MR            # 2048 batch1 rows per core
K = B // MC               # 1024 batch2 rows per core
MB = ROWS // 128          # 16 i-blocks
KB = K // 128             # 8 k-blocks
CC = C // 128             # 4 contraction chunks
TEMP = 0.1
N_TERMS = B * (B - 1) // 2
ACT_TABLE_LN_EXP = 6      # natural_log_exp_and_others in act_info.json

_CACHE = {}

CFG = {}


def build_bass():
    import concourse.bass as bass
    import concourse.bacc as bacc
    import concourse.tile as tile
    from concourse import mybir
    from contextlib import ExitStack

    fp32 = mybir.dt.float32
    fp8 = mybir.dt.float8e4
    AF = mybir.ActivationFunctionType
    ALU = mybir.AluOpType
    PM = mybir.MatmulPerfMode

    nc = bacc.Bacc("TRN2", target_bir_lowering=False, debug=False,
                   num_devices=NCORES)

    b1t = nc.dram_tensor("b1t", [C, ROWS], fp8, kind="ExternalInput")
    b1n_d = nc.dram_tensor("b1n", [ROWS, C], fp8, kind="ExternalInput")
    b2n_d = nc.dram_tensor("b2n", [K, C], fp8, kind="ExternalInput")
    ident = nc.dram_tensor("ident", [128, 128], fp8, kind="ExternalInput")
    out = nc.dram_tensor("out", [128, MB + CC], fp32, kind="ExternalOutput")

    with tile.TileContext(nc) as tc, ExitStack() as ctx:
        sb = ctx.enter_context(tc.tile_pool(name="sb", bufs=1))
        dumps = ctx.enter_context(tc.tile_pool(name="dumps", bufs=3))
        pt = ctx.enter_context(tc.tile_pool(name="pt", bufs=2, space="PSUM"))
        pneg = ctx.enter_context(tc.tile_pool(name="pneg", bufs=3, space="PSUM"))

        b1T = sb.tile([128, CC, ROWS], fp8, name="b1T")
        b1n = sb.tile([128, MB, C], fp8, name="b1n")
        b2n = sb.tile([128, KB, C], fp8, name="b2n")
        identb = sb.tile([128, 128], fp8, name="identb")
        b2sT = sb.tile([128, CC, K], fp8, name="b2sT")
        diag2 = sb.tile([128, KB, 128], fp8, name="diag2")
        ssq1 = sb.tile([128, MB], fp32, name="ssq1")
        ssq2 = sb.tile([128, KB], fp32, name="ssq2")
        ln1 = sb.tile([128, MB], fp32, name="ln1")
        ln2 = sb.tile([128, KB], fp32, name="ln2")
        invn1 = sb.tile([128, MB], fp32, name="invn1")
        invn1f8 = sb.tile([128, MB], fp8, name="invn1f8")
        invn2s = sb.tile([128, KB], fp32, name="invn2s")
        stage = sb.tile([128, MB + CC], fp32, name="stage")

        # single ACT table that serves Exp/Ln/Copy/Square for the whole kernel
        nc.scalar.add_instruction(mybir.InstLoadActFuncSet(
            name=nc.get_next_instruction_name(), ins=[], outs=[],
            act_func_set_id=ACT_TABLE_LN_EXP))

        # ---- input DMAs (gpsimd SWDGE: lowest issue latency; b2n first) ------
        nc.gpsimd.dma_start(
            b2n[:, :, :], b2n_d.ap().rearrange("(kb p) c -> p kb c", p=128))
        nc.gpsimd.dma_start(
            b1n[:, :, :], b1n_d.ap().rearrange("(mb p) c -> p mb c", p=128))
        nc.gpsimd.dma_start(
            b1T[:, :, :], b1t.ap().rearrange("(cc p) i -> p cc i", p=128))
        nc.sync.dma_start(identb[:, :], ident.ap())

        # ---- batch2 path (streamed in 2 groups of 4 k-blocks): --------------
        # ssq (split DVE/ACT) -> rsqrt(x10) -> diag (ACT) -> PE transpose
        # -> evac (DVE cast fp8)
        def ssq2_block(kb, eng):
            if eng == "dve":
                dmp = dumps.tile([128, C], fp8, name="dssq2", tag="dssq2")
                nc.vector.scalar_tensor_tensor(
                    out=dmp[:, :], in0=b2n[:, kb, :], scalar=1.0,
                    in1=b2n[:, kb, :], op0=ALU.mult, op1=ALU.mult,
                    accum_out=ssq2[:, kb:kb + 1])
            else:
                dmp = dumps.tile([128, C], fp8, name="assq2", tag="assq2")
                nc.scalar.activation(
                    dmp[:, :], b2n[:, kb, :], AF.Square,
                    accum_out=ssq2[:, kb:kb + 1])

        def transpose_block(kb):
            ptile = pt.tile([128, CC, 128], fp32, name="ptile", tag="pt")
            for cc in range(CC):
                nc.tensor.matmul(
                    ptile[:, cc, :],
                    lhsT=b2n[:, kb, cc * 128:(cc + 1) * 128],
                    rhs=diag2[:, kb, :],
                    start=True, stop=True)
            nc.vector.tensor_copy(
                b2sT[:, :, kb * 128:(kb + 1) * 128], ptile[:, :, :])

        def ssq1_block(mb):
            dmp = dumps.tile([128, C], fp8, name="dssq1", tag="dssq1")
            nc.vector.scalar_tensor_tensor(
                out=dmp[:, :], in0=b1n[:, mb, :], scalar=1.0,
                in1=b1n[:, mb, :], op0=ALU.mult, op1=ALU.mult,
                accum_out=ssq1[:, mb:mb + 1])

        for g in range(2):
            gs = slice(g * 4, (g + 1) * 4)
            # DVE takes the first two blocks, ACT the other two (in parallel)
            ssq2_block(g * 4 + 0, "dve")
            ssq2_block(g * 4 + 2, "act")
            ssq2_block(g * 4 + 1, "dve")
            ssq2_block(g * 4 + 3, "act")
            # 10/sqrt(x) == exp(-0.5 * ln(0.01 * x)); 10 = 1/TEMP
            nc.scalar.activation(ln2[:, gs], ssq2[:, gs], AF.Ln, scale=0.01)
            nc.scalar.activation(invn2s[:, gs], ln2[:, gs], AF.Exp, scale=-0.5)
            for kb in range(g * 4, (g + 1) * 4):
                # diag on ACT: Copy computes out = in * scale  (table-safe)
                nc.scalar.activation(
                    diag2[:, kb, :], identb[:, :], AF.Copy,
                    scale=invn2s[:, kb:kb + 1])
            for kb in range(g * 4, (g + 1) * 4):
                transpose_block(kb)

        # ---- batch1 stats: ssq on DVE (after the b2-path evacs) -------------
        for mb in range(MB):
            ssq1_block(mb)

        def rsqrt1_granule(g):
            gs = slice(g * 4, (g + 1) * 4)
            nc.scalar.activation(ln1[:, gs], ssq1[:, gs], AF.Ln)
            nc.scalar.activation(invn1[:, gs], ln1[:, gs], AF.Exp, scale=-0.5)

        # ---- main: neg strip matmul (fp8 DoubleRow) + fused exp-rowsum ------
        # rsqrt granules interleave with the exp stream (ACT executes
        # in-order): granule g lands right before exp of i-block 4g.
        for m in range(MB):
            if m % 4 == 0:
                rsqrt1_granule(m // 4)
            ntile = pneg.tile([128, 2, 512], fp32, name="ntile", tag="pneg")
            for kg in range(2):
                for mg in range(2):
                    nc.tensor.matmul(
                        ntile[:, mg, :],
                        lhsT=b1T[:, 2 * kg:2 * kg + 2, m * 128:(m + 1) * 128],
                        rhs=b2sT[:, 2 * kg:2 * kg + 2, mg * 512:(mg + 1) * 512],
                        start=(kg == 0), stop=(kg == 1),
                        perf_mode=PM.DoubleRow)
            dmp = dumps.tile([128, 1024], fp8, name="dexp", tag="dexp")
            nc.scalar.activation(
                dmp[:, :], ntile[:, :, :].rearrange("p a b -> p (a b)"),
                AF.Exp, scale=invn1[:, m:m + 1],
                accum_out=stage[:, m:m + 1])

        # ---- s partial: s[c] = sum_i b1[i,c] * inv1[i] over this strip ------
        nc.vector.tensor_copy(invn1f8[:, :], invn1[:, :])
        psum_s = pt.tile([128, CC], fp32, name="psum_s", tag="pt")
        for cc in range(CC):
            for mb in range(MB):
                nc.tensor.matmul(
                    psum_s[:, cc:cc + 1],
                    lhsT=b1n[:, mb, cc * 128:(cc + 1) * 128],
                    rhs=invn1f8[:, mb:mb + 1],
                    start=(mb == 0), stop=(mb == MB - 1))
        nc.vector.tensor_copy(stage[:, MB:MB + CC], psum_s[:, :])

        nc.sync.dma_start(out.ap(), stage[:, :])

    nc.compile()
    return nc


def _get_nc():
    key = ("nc", tuple(sorted(CFG.items())))
    if key not in _CACHE:
        _CACHE[key] = build_bass()
    return _CACHE[key]


def make_in_maps(batch1, batch2):
    f8 = ml_dtypes.float8_e4m3
    batch1 = np.ascontiguousarray(np.asarray(batch1, dtype=np.float32))
    batch2 = np.ascontiguousarray(np.asarray(batch2, dtype=np.float32))
    eye = np.eye(128, dtype=f8)
    maps = []
    b1s = []
    for r in range(MR):
        strip = batch1[r * ROWS:(r + 1) * ROWS]
        b1s.append({
            "b1t": np.ascontiguousarray(strip.T.astype(f8)),
            "b1n": np.ascontiguousarray(strip.astype(f8)),
        })
    b2s = [np.ascontiguousarray(batch2[c * K:(c + 1) * K].astype(f8))
           for c in range(MC)]
    for core in range(NCORES):
        r, c = divmod(core, MC)
        maps.append({
            "b1t": b1s[r]["b1t"], "b1n": b1s[r]["b1n"],
            "b2n": b2s[c], "ident": eye,
        })
    return maps


def combine(results):
    """Host-side gather.  results[core]["out"] is [128, MB+CC] fp32:
    cols 0..MB-1 = D partials (row i = m*128 + p of the core's strip),
    cols MB..    = s partial [c split over (cc, p)]."""
    # ld: sum the 4 col-group partials per row-group, then log
    ld = np.empty(B, dtype=np.float64)
    for r in range(MR):
        d = np.zeros((128, MB), dtype=np.float64)
        for c in range(MC):
            d += np.asarray(results[r * MC + c]["out"][:, :MB], np.float64)
        # row index within strip = m*128 + p  ->  [MB, 128] transposed flat
        ld[r * ROWS:(r + 1) * ROWS] = np.log(d.T.reshape(-1))
    # s: each row-group leader computed the full strip partial; sum groups
    s = np.zeros(C, dtype=np.float64)
    for r in range(MR):
        sp = np.asarray(results[r * MC]["out"][:, MB:MB + CC], np.float64)
        s += sp.T.reshape(-1)  # c = cc*128 + p
    term1 = np.dot(np.arange(B, dtype=np.float64), ld)
    tri = (np.dot(s, s) / TEMP - B / TEMP) / 2.0
    return np.asarray((term1 - tri) / N_TERMS, dtype=np.float32)


def run_hw(in_maps, trace=False, **kwargs):
    from concourse.bass_utils import run_bass_kernel_spmd
    return run_bass_kernel_spmd(_get_nc(), in_maps,
                                core_ids=list(range(NCORES)),
                                trace=trace, **kwargs)


def kernel(batch1, batch2):
    res = run_hw(make_in_maps(batch1, batch2))
    return combine(res.results)


# revision 11
# speedup vs baseline: 1.2217x; 1.2217x over previous
"""Trainium2 Bass kernel for nn_DistanceLoss (contrastive loss over cosine
similarity matrices).

Math restructure (vs the reference):
  loss = [ sum_i i*ld[i] - sum_{i>j} pos[i,j] ] / n_terms
where ld = logsumexp_k(neg[i,k]).  pos = (p1 @ p1.T)/T is symmetric with
diagonal 1/T, so the strict-lower-triangular sum collapses to
  ( ||sum_i p1_i||^2 / T - B/T ) / 2,
needing only the column-sum s of normalized batch1.  Only
neg = p1n @ p2n.T needs real compute.

Sharding: 2x4 grid.  Row-groups r=0,1 split batch1 rows (2048 each);
col-groups c=0..3 split batch2 rows (1024 each).  Core = r*4 + c computes a
[2048, 1024] block of neg and emits partial denominators
D[i] = sum_{k in slice} exp(neg[i,k]); the host sums the 4 partials per
row-group, takes log, and does the final tiny reduction in float64.

Host-side prep is layout/cast only: fp8e4 casts and a pre-transposed copy
of the batch1 strip (b1T) so the device does zero b1-side transposes.  All
normalization math stays on device:
  - ssq/rsqrt of both batches on device (DVE/GpSimd STT + ACT Ln/Exp)
  - batch2 rows are normalized (x10 = 1/TEMP) during the PE diag-transpose
  - batch1 rows are normalized by folding inv1[i] into the ACT Exp *scale
    vector* (per-partition AP) -- the main matmul consumes raw fp8 b1T.
Main matmul runs fp8 DoubleRow (2 c-chunks per pass).  A single manual
ACT table load (natural_log_exp_and_others serves Exp/Ln/Copy/Square)
avoids the per-switch 1.28us table reloads.
"""

import numpy as np
import ml_dtypes

B = 4096
C = 512
NCORES = 8
MR = 2                    # row groups (batch1 split)
MC = 4                    # col groups (batch2 split)
ROWS = B // MR            # 2048 batch1 rows per core
K = B // MC               # 1024 batch2 rows per core
MB = ROWS // 128          # 16 i-blocks
KB = K // 128             # 8 k-blocks
CC = C // 128             # 4 contraction chunks
TEMP = 0.1
N_TERMS = B * (B - 1) // 2
ACT_TABLE_LN_EXP = 6      # natural_log_exp_and_others in act_info.json

_CACHE = {}

CFG = {}


def build_bass():
    import concourse.bass as bass
    import concourse.bacc as bacc
    import concourse.tile as tile
    from concourse import mybir
    from contextlib import ExitStack

    fp32 = mybir.dt.float32
    fp8 = mybir.dt.float8e4
    AF = mybir.ActivationFunctionType
    ALU = mybir.AluOpType
    PM = mybir.MatmulPerfMode

    nc = bacc.Bacc("TRN2", target_bir_lowering=False, debug=False,
                   num_devices=NCORES)

    b1t = nc.dram_tensor("b1t", [C, ROWS], fp8, kind="ExternalInput")
    b1n_d = nc.dram_tensor("b1n", [ROWS, C], fp8, kind="ExternalInput")
    b2n_d = nc.dram_tensor("b2n", [K, C], fp8, kind="ExternalInput")
    ident = nc.dram_tensor("ident", [128, 128], fp8, kind="ExternalInput")
    out = nc.dram_tensor("out", [128, MB + CC], fp32, kind="ExternalOutput")

    with tile.TileContext(nc) as tc, ExitStack() as ctx:
        sb = ctx.enter_context(tc.tile_pool(name="sb", bufs=1))
        dumps = ctx.enter_context(tc.tile_pool(name="dumps", bufs=3))
        pt = ctx.enter_context(tc.tile_pool(name="pt", bufs=2, space="PSUM"))
        pneg = ctx.enter_context(tc.tile_pool(name="pneg", bufs=3, space="PSUM"))

        b1T = sb.tile([128, CC, ROWS], fp8, name="b1T")
        b1n = sb.tile([128, MB, C], fp8, name="b1n")
        b2n = sb.tile([128, KB, C], fp8, name="b2n")
        identb = sb.tile([128, 128], fp8, name="identb")
        b2sT = sb.tile([128, CC, K], fp8, name="b2sT")
        diag2 = sb.tile([128, KB, 128], fp8, name="diag2")
        ssq1 = sb.tile([128, MB], fp32, name="ssq1")
        ssq2 = sb.tile([128, KB], fp32, name="ssq2")
        ln1 = sb.tile([128, MB], fp32, name="ln1")
        ln2 = sb.tile([128, KB], fp32, name="ln2")
        invn1 = sb.tile([128, MB], fp32, name="invn1")
        invn1f8 = sb.tile([128, MB], fp8, name="invn1f8")
        invn2s = sb.tile([128, KB], fp32, name="invn2s")
        stage = sb.tile([128, MB + CC], fp32, name="stage")

        # single ACT table that serves Exp/Ln/Copy/Square for the whole kernel
        nc.scalar.add_instruction(mybir.InstLoadActFuncSet(
            name=nc.get_next_instruction_name(), ins=[], outs=[],
            act_func_set_id=ACT_TABLE_LN_EXP))

        # ---- input DMAs (gpsimd SWDGE: lowest issue latency; b2n first, in
        # two chunks so ssq2 of the first k-blocks starts sooner) -------------
        b2ap = b2n_d.ap().rearrange("(kb p) c -> p kb c", p=128)
        nc.gpsimd.dma_start(b2n[:, 0:4, :], b2ap[:, 0:4, :])
        nc.gpsimd.dma_start(b2n[:, 4:KB, :], b2ap[:, 4:KB, :])
        nc.gpsimd.dma_start(
            b1n[:, :, :], b1n_d.ap().rearrange("(mb p) c -> p mb c", p=128))
        nc.gpsimd.dma_start(
            b1T[:, :, :], b1t.ap().rearrange("(cc p) i -> p cc i", p=128))
        nc.sync.dma_start(identb[:, :], ident.ap())

        # ---- batch2 path (streamed in 2 groups of 4 k-blocks): --------------
        # ssq (split DVE/ACT) -> rsqrt(x10) -> diag (ACT) -> PE transpose
        # -> evac (DVE cast fp8)
        def ssq2_block(kb):
            dmp = dumps.tile([128, C], fp8, name="dssq2", tag="dssq2")
            nc.vector.scalar_tensor_tensor(
                out=dmp[:, :], in0=b2n[:, kb, :], scalar=1.0,
                in1=b2n[:, kb, :], op0=ALU.mult, op1=ALU.mult,
                accum_out=ssq2[:, kb:kb + 1])

        def transpose_block(kb):
            ptile = pt.tile([128, CC, 128], fp32, name="ptile", tag="pt")
            for cc in range(CC):
                nc.tensor.matmul(
                    ptile[:, cc, :],
                    lhsT=b2n[:, kb, cc * 128:(cc + 1) * 128],
                    rhs=diag2[:, kb, :],
                    start=True, stop=True)
            # evac on ACT: Copy (in the loaded table) is cheaper there, and
            # ACT is otherwise idle before the exp stream
            nc.scalar.copy(b2sT[:, :, kb * 128:(kb + 1) * 128], ptile[:, :, :])

        def ssq1_block(mb):
            dmp = dumps.tile([128, C], fp8, name="dssq1", tag="dssq1")
            nc.vector.scalar_tensor_tensor(
                out=dmp[:, :], in0=b1n[:, mb, :], scalar=1.0,
                in1=b1n[:, mb, :], op0=ALU.mult, op1=ALU.mult,
                accum_out=ssq1[:, mb:mb + 1])

        for g in range(2):
            gs = slice(g * 4, (g + 1) * 4)
            for kb in range(g * 4, (g + 1) * 4):
                ssq2_block(kb)
            # 10/sqrt(x) == exp(-0.5 * ln(0.01 * x)); 10 = 1/TEMP
            nc.scalar.activation(ln2[:, gs], ssq2[:, gs], AF.Ln, scale=0.01)
            nc.scalar.activation(invn2s[:, gs], ln2[:, gs], AF.Exp, scale=-0.5)
            for kb in range(g * 4, (g + 1) * 4):
                nc.vector.tensor_scalar_mul(
                    diag2[:, kb, :], identb[:, :], invn2s[:, kb:kb + 1])
            for kb in range(g * 4, (g + 1) * 4):
                transpose_block(kb)

        # ---- batch1 stats: ssq on DVE (after the b2-path evacs) -------------
        for mb in range(MB):
            ssq1_block(mb)

        def rsqrt1_granule(g):
            gs = slice(g * 4, (g + 1) * 4)
            nc.scalar.activation(ln1[:, gs], ssq1[:, gs], AF.Ln)
            nc.scalar.activation(invn1[:, gs], ln1[:, gs], AF.Exp, scale=-0.5)

        # ---- main: neg strip matmul (fp8 DoubleRow) + fused exp-rowsum ------
        # rsqrt granules interleave with the exp stream (ACT executes
        # in-order): granule g lands right before exp of i-block 4g.
        for m in range(MB):
            if m % 4 == 0:
                rsqrt1_granule(m // 4)
            ntile = pneg.tile([128, 2, 512], fp32, name="ntile", tag="pneg")
            for kg in range(2):
                for mg in range(2):
                    nc.tensor.matmul(
                        ntile[:, mg, :],
                        lhsT=b1T[:, 2 * kg:2 * kg + 2, m * 128:(m + 1) * 128],
                        rhs=b2sT[:, 2 * kg:2 * kg + 2, mg * 512:(mg + 1) * 512],
                        start=(kg == 0), stop=(kg == 1),
                        perf_mode=PM.DoubleRow)
            dmp = dumps.tile([128, 1024], fp8, name="dexp", tag="dexp")
            nc.scalar.activation(
                dmp[:, :], ntile[:, :, :].rearrange("p a b -> p (a b)"),
                AF.Exp, scale=invn1[:, m:m + 1],
                accum_out=stage[:, m:m + 1])

        # ---- s partial: s[c] = sum_i b1[i,c] * inv1[i] over this strip ------
        nc.vector.tensor_copy(invn1f8[:, :], invn1[:, :])
        psum_s = pt.tile([128, CC], fp32, name="psum_s", tag="pt")
        for cc in range(CC):
            for mb in range(MB):
                nc.tensor.matmul(
                    psum_s[:, cc:cc + 1],
                    lhsT=b1n[:, mb, cc * 128:(cc + 1) * 128],
                    rhs=invn1f8[:, mb:mb + 1],
                    start=(mb == 0), stop=(mb == MB - 1))
        nc.vector.tensor_copy(stage[:, MB:MB + CC], psum_s[:, :])

        nc.sync.dma_start(out.ap(), stage[:, :])

    nc.compile()
    return nc


def _get_nc():
    key = ("nc", tuple(sorted(CFG.items())))
    if key not in _CACHE:
        _CACHE[key] = build_bass()
    return _CACHE[key]


def make_in_maps(batch1, batch2):
    f8 = ml_dtypes.float8_e4m3
    batch1 = np.ascontiguousarray(np.asarray(batch1, dtype=np.float32))
    batch2 = np.ascontiguousarray(np.asarray(batch2, dtype=np.float32))
    eye = np.eye(128, dtype=f8)
    maps = []
    b1s = []
    for r in range(MR):
        strip = batch1[r * ROWS:(r + 1) * ROWS]
        b1s.append({
            "b1t": np.ascontiguousarray(strip.T.astype(f8)),
            "b1n": np.ascontiguousarray(strip.astype(f8)),
        })
    b2s = [np.ascontiguousarray(batch2[c * K:(c + 1) * K].astype(f8))
           for c in range(MC)]
    for core in range(NCORES):
        r, c = divmod(core, MC)
        maps.append({
            "b1t": b1s[r]["b1t"], "b1n": b1s[r]["b1n"],
            "b2n": b2s[c], "ident": eye,
        })
    return maps


def combine(results):
    """Host-side gather.  results[core]["out"] is [128, MB+CC] fp32:
    cols 0..MB-1 = D partials (row i = m*128 + p of the core's strip),
    cols MB..    = s partial [c split over (cc, p)]."""
    # ld: sum the 4 col-group partials per row-group, then log
    ld = np.empty(B, dtype=np.float64)
    for r in range(MR):
        d = np.zeros((128, MB), dtype=np.float64)
        for c in range(MC):
            d += np.asarray(results[r * MC + c]["out"][:, :MB], np.float64)
        # row index within strip = m*128 + p  ->  [MB, 128] transposed flat
        ld[r * ROWS:(r + 1) * ROWS] = np.log(d.T.reshape(-1))
    # s: each row-group leader computed the full strip partial; sum groups
    s = np.zeros(C, dtype=np.float64)
    for r in range(MR):
        sp = np.asarray(results[r * MC]["out"][:, MB:MB + CC], np.float64)
        s += sp.T.reshape(-1)  # c = cc*128 + p
    term1 = np.dot(np.arange(B, dtype=np.float64), ld)
    tri = (np.dot(s, s) / TEMP - B / TEMP) / 2.0
    return np.asarray((term1 - tri) / N_TERMS, dtype=np.float32)


def run_hw(in_maps, trace=False, **kwargs):
    from concourse.bass_utils import run_bass_kernel_spmd
    return run_bass_kernel_spmd(_get_nc(), in_maps,
                                core_ids=list(range(NCORES)),
                                trace=trace, **kwargs)


def kernel(batch1, batch2):
    res = run_hw(make_in_maps(batch1, batch2))
    return combine(res.results)


# revision 14
# speedup vs baseline: 1.2232x; 1.0012x over previous
"""Trainium2 Bass kernel for nn_DistanceLoss (contrastive loss over cosine
similarity matrices).

Math restructure (vs the reference):
  loss = [ sum_i i*ld[i] - sum_{i>j} pos[i,j] ] / n_terms
where ld = logsumexp_k(neg[i,k]).  pos = (p1 @ p1.T)/T is symmetric with
diagonal 1/T, so the strict-lower-triangular sum collapses to
  ( ||sum_i p1_i||^2 / T - B/T ) / 2,
needing only the column-sum s of normalized batch1.  Only
neg = p1n @ p2n.T needs real compute.

Sharding: 2x4 grid.  Row-groups r=0,1 split batch1 rows (2048 each);
col-groups c=0..3 split batch2 rows (1024 each).  Core = r*4 + c computes a
[2048, 1024] block of neg and emits partial denominators
D[i] = sum_{k in slice} exp(neg[i,k]); the host sums the 4 partials per
row-group, takes log, and does the final tiny reduction in float64.

Host-side prep is layout/cast only: fp8e4 casts and a pre-transposed copy
of the batch1 strip (b1T) so the device does zero b1-side transposes.  All
normalization math stays on device:
  - ssq/rsqrt of both batches on device (DVE/GpSimd STT + ACT Ln/Exp)
  - batch2 rows are normalized (x10 = 1/TEMP) during the PE diag-transpose
  - batch1 rows are normalized by folding inv1[i] into the ACT Exp *scale
    vector* (per-partition AP) -- the main matmul consumes raw fp8 b1T.
Main matmul runs fp8 DoubleRow (2 c-chunks per pass).  A single manual
ACT table load (natural_log_exp_and_others serves Exp/Ln/Copy/Square)
avoids the per-switch 1.28us table reloads.
"""

import numpy as np
import ml_dtypes

B = 4096
C = 512
NCORES = 8
MR = 2                    # row groups (batch1 split)
MC = 4                    # col groups (batch2 split)
ROWS = B // MR            # 2048 batch1 rows per core
K = B // MC               # 1024 batch2 rows per core
MB = ROWS // 128          # 16 i-blocks
KB = K // 128             # 8 k-blocks
CC = C // 128             # 4 contraction chunks
TEMP = 0.1
N_TERMS = B * (B - 1) // 2
ACT_TABLE_LN_EXP = 6      # natural_log_exp_and_others in act_info.json

_CACHE = {}

CFG = {}


def build_bass():
    import concourse.bass as bass
    import concourse.bacc as bacc
    import concourse.tile as tile
    from concourse import mybir
    from contextlib import ExitStack

    fp32 = mybir.dt.float32
    fp8 = mybir.dt.float8e4
    AF = mybir.ActivationFunctionType
    ALU = mybir.AluOpType
    PM = mybir.MatmulPerfMode

    nc = bacc.Bacc("TRN2", target_bir_lowering=False, debug=False,
                   num_devices=NCORES)

    b1t = nc.dram_tensor("b1t", [C, ROWS], fp8, kind="ExternalInput")
    b1n_d = nc.dram_tensor("b1n", [ROWS, C], fp8, kind="ExternalInput")
    b2n_d = nc.dram_tensor("b2n", [K, C], fp8, kind="ExternalInput")
    ident = nc.dram_tensor("ident", [128, 128], fp8, kind="ExternalInput")
    out = nc.dram_tensor("out", [128, MB + CC], fp32, kind="ExternalOutput")

    with tile.TileContext(nc) as tc, ExitStack() as ctx:
        sb = ctx.enter_context(tc.tile_pool(name="sb", bufs=1))
        dumps = ctx.enter_context(tc.tile_pool(name="dumps", bufs=3))
        pt = ctx.enter_context(tc.tile_pool(name="pt", bufs=2, space="PSUM"))
        pneg = ctx.enter_context(tc.tile_pool(name="pneg", bufs=3, space="PSUM"))

        b1T = sb.tile([128, CC, ROWS], fp8, name="b1T")
        b1n = sb.tile([128, MB, C], fp8, name="b1n")
        b2n = sb.tile([128, KB, C], fp8, name="b2n")
        identb = sb.tile([128, 128], fp8, name="identb")
        b2sT = sb.tile([128, CC, K], fp8, name="b2sT")
        diag2 = sb.tile([128, KB, 128], fp8, name="diag2")
        ssq1 = sb.tile([128, MB], fp32, name="ssq1")
        ssq2 = sb.tile([128, KB], fp32, name="ssq2")
        ln1 = sb.tile([128, MB], fp32, name="ln1")
        ln2 = sb.tile([128, KB], fp32, name="ln2")
        invn1 = sb.tile([128, MB], fp32, name="invn1")
        invn1f8 = sb.tile([128, MB], fp8, name="invn1f8")
        invn2s = sb.tile([128, KB], fp32, name="invn2s")
        stage = sb.tile([128, MB + CC], fp32, name="stage")

        # single ACT table that serves Exp/Ln/Copy/Square for the whole kernel
        nc.scalar.add_instruction(mybir.InstLoadActFuncSet(
            name=nc.get_next_instruction_name(), ins=[], outs=[],
            act_func_set_id=ACT_TABLE_LN_EXP))

        # ---- input DMAs (gpsimd SWDGE: lowest issue latency; b2n first, in
        # two chunks so ssq2 of the first k-blocks starts sooner) -------------
        b2ap = b2n_d.ap().rearrange("(kb p) c -> p kb c", p=128)
        nc.gpsimd.dma_start(b2n[:, 0:4, :], b2ap[:, 0:4, :])
        nc.gpsimd.dma_start(b2n[:, 4:KB, :], b2ap[:, 4:KB, :])
        nc.gpsimd.dma_start(
            b1n[:, :, :], b1n_d.ap().rearrange("(mb p) c -> p mb c", p=128))
        nc.gpsimd.dma_start(
            b1T[:, :, :], b1t.ap().rearrange("(cc p) i -> p cc i", p=128))
        nc.sync.dma_start(identb[:, :], ident.ap())

        # ---- batch2 path (streamed in 2 groups of 4 k-blocks): --------------
        # ssq (split DVE/ACT) -> rsqrt(x10) -> diag (ACT) -> PE transpose
        # -> evac (DVE cast fp8)
        def ssq2_block(kb):
            dmp = dumps.tile([128, C], fp8, name="dssq2", tag="dssq2")
            nc.vector.scalar_tensor_tensor(
                out=dmp[:, :], in0=b2n[:, kb, :], scalar=1.0,
                in1=b2n[:, kb, :], op0=ALU.mult, op1=ALU.mult,
                accum_out=ssq2[:, kb:kb + 1])

        def transpose_block(kb, evac_eng):
            ptile = pt.tile([128, CC, 128], fp32, name="ptile", tag="pt")
            for cc in range(CC):
                nc.tensor.matmul(
                    ptile[:, cc, :],
                    lhsT=b2n[:, kb, cc * 128:(cc + 1) * 128],
                    rhs=diag2[:, kb, :],
                    start=True, stop=True)
            # evac split across DVE and ACT so the two groups drain in parallel
            if evac_eng == "act":
                nc.scalar.copy(
                    b2sT[:, :, kb * 128:(kb + 1) * 128], ptile[:, :, :])
            else:
                nc.vector.tensor_copy(
                    b2sT[:, :, kb * 128:(kb + 1) * 128], ptile[:, :, :])

        def ssq1_block(mb):
            dmp = dumps.tile([128, C], fp8, name="dssq1", tag="dssq1")
            nc.vector.scalar_tensor_tensor(
                out=dmp[:, :], in0=b1n[:, mb, :], scalar=1.0,
                in1=b1n[:, mb, :], op0=ALU.mult, op1=ALU.mult,
                accum_out=ssq1[:, mb:mb + 1])

        # all ssq2 first (dense DVE stream, overlapping the DMA chunks), with
        # the rsqrt pairs (ACT) slotted per granule; then both diag groups,
        # then the transposes with evacs split ACT (g0) / DVE (g1)
        for g in range(2):
            gs = slice(g * 4, (g + 1) * 4)
            for kb in range(g * 4, (g + 1) * 4):
                ssq2_block(kb)
            # 10/sqrt(x) == exp(-0.5 * ln(0.01 * x)); 10 = 1/TEMP
            nc.scalar.activation(ln2[:, gs], ssq2[:, gs], AF.Ln, scale=0.01)
            nc.scalar.activation(invn2s[:, gs], ln2[:, gs], AF.Exp, scale=-0.5)
        for kb in range(KB):
            nc.vector.tensor_scalar_mul(
                diag2[:, kb, :], identb[:, :], invn2s[:, kb:kb + 1])
        for kb in range(KB):
            transpose_block(kb, "act")

        # ---- batch1 stats: ssq on DVE (after the b2-path evacs) -------------
        for mb in range(MB):
            ssq1_block(mb)

        def rsqrt1_granule(g):
            gs = slice(g * 4, (g + 1) * 4)
            nc.scalar.activation(ln1[:, gs], ssq1[:, gs], AF.Ln)
            nc.scalar.activation(invn1[:, gs], ln1[:, gs], AF.Exp, scale=-0.5)

        # ---- main: neg strip matmul (fp8 DoubleRow) + fused exp-rowsum ------
        # rsqrt granules interleave with the exp stream (ACT executes
        # in-order): granule g lands right before exp of i-block 4g.
        for m in range(MB):
            if m % 4 == 0:
                rsqrt1_granule(m // 4)
            ntile = pneg.tile([128, 2, 512], fp32, name="ntile", tag="pneg")
            for kg in range(2):
                for mg in range(2):
                    nc.tensor.matmul(
                        ntile[:, mg, :],
                        lhsT=b1T[:, 2 * kg:2 * kg + 2, m * 128:(m + 1) * 128],
                        rhs=b2sT[:, 2 * kg:2 * kg + 2, mg * 512:(mg + 1) * 512],
                        start=(kg == 0), stop=(kg == 1),
                        perf_mode=PM.DoubleRow)
            dmp = dumps.tile([128, 1024], fp8, name="dexp", tag="dexp")
            nc.scalar.activation(
                dmp[:, :], ntile[:, :, :].rearrange("p a b -> p (a b)"),
                AF.Exp, scale=invn1[:, m:m + 1],
                accum_out=stage[:, m:m + 1])

        # ---- s partial: s[c] = sum_i b1[i,c] * inv1[i] over this strip ------
        nc.vector.tensor_copy(invn1f8[:, :], invn1[:, :])
        psum_s = pt.tile([128, CC], fp32, name="psum_s", tag="pt")
        for cc in range(CC):
            for mb in range(MB):
                nc.tensor.matmul(
                    psum_s[:, cc:cc + 1],
                    lhsT=b1n[:, mb, cc * 128:(cc + 1) * 128],
                    rhs=invn1f8[:, mb:mb + 1],
                    start=(mb == 0), stop=(mb == MB - 1))
        nc.vector.tensor_copy(stage[:, MB:MB + CC], psum_s[:, :])

        nc.sync.dma_start(out.ap(), stage[:, :])

    nc.compile()
    return nc


def _get_nc():
    key = ("nc", tuple(sorted(CFG.items())))
    if key not in _CACHE:
        _CACHE[key] = build_bass()
    return _CACHE[key]


def make_in_maps(batch1, batch2):
    f8 = ml_dtypes.float8_e4m3
    batch1 = np.ascontiguousarray(np.asarray(batch1, dtype=np.float32))
    batch2 = np.ascontiguousarray(np.asarray(batch2, dtype=np.float32))
    eye = np.eye(128, dtype=f8)
    maps = []
    b1s = []
    for r in range(MR):
        strip = batch1[r * ROWS:(r + 1) * ROWS]
        b1s.append({
            "b1t": np.ascontiguousarray(strip.T.astype(f8)),
            "b1n": np.ascontiguousarray(strip.astype(f8)),
        })
    b2s = [np.ascontiguousarray(batch2[c * K:(c + 1) * K].astype(f8))
           for c in range(MC)]
    for core in range(NCORES):
        r, c = divmod(core, MC)
        maps.append({
            "b1t": b1s[r]["b1t"], "b1n": b1s[r]["b1n"],
            "b2n": b2s[c], "ident": eye,
        })
    return maps


def combine(results):
    """Host-side gather.  results[core]["out"] is [128, MB+CC] fp32:
    cols 0..MB-1 = D partials (row i = m*128 + p of the core's strip),
    cols MB..    = s partial [c split over (cc, p)]."""
    # ld: sum the 4 col-group partials per row-group, then log
    ld = np.empty(B, dtype=np.float64)
    for r in range(MR):
        d = np.zeros((128, MB), dtype=np.float64)
        for c in range(MC):
            d += np.asarray(results[r * MC + c]["out"][:, :MB], np.float64)
        # row index within strip = m*128 + p  ->  [MB, 128] transposed flat
        ld[r * ROWS:(r + 1) * ROWS] = np.log(d.T.reshape(-1))
    # s: each row-group leader computed the full strip partial; sum groups
    s = np.zeros(C, dtype=np.float64)
    for r in range(MR):
        sp = np.asarray(results[r * MC]["out"][:, MB:MB + CC], np.float64)
        s += sp.T.reshape(-1)  # c = cc*128 + p
    term1 = np.dot(np.arange(B, dtype=np.float64), ld)
    tri = (np.dot(s, s) / TEMP - B / TEMP) / 2.0
    return np.asarray((term1 - tri) / N_TERMS, dtype=np.float32)


def run_hw(in_maps, trace=False, **kwargs):
    from concourse.bass_utils import run_bass_kernel_spmd
    return run_bass_kernel_spmd(_get_nc(), in_maps,
                                core_ids=list(range(NCORES)),
                                trace=trace, **kwargs)


def kernel(batch1, batch2):
    res = run_hw(make_in_maps(batch1, batch2))
    return combine(res.results)


# revision 19
# speedup vs baseline: 1.2911x; 1.0555x over previous
"""Trainium2 Bass kernel for nn_DistanceLoss (contrastive loss over cosine
similarity matrices).

Math restructure (vs the reference):
  loss = [ sum_i i*ld[i] - sum_{i>j} pos[i,j] ] / n_terms
where ld = logsumexp_k(neg[i,k]).  pos = (p1 @ p1.T)/T is symmetric with
diagonal 1/T, so the strict-lower-triangular sum collapses to
  ( ||sum_i p1_i||^2 / T - B/T ) / 2,
needing only the column-sum s of normalized batch1.  Only
neg = p1n @ p2n.T needs real compute.

Sharding: 2x4 grid.  Row-groups r=0,1 split batch1 rows (2048 each);
col-groups c=0..3 split batch2 rows (1024 each).  Core = r*4 + c computes a
[2048, 1024] block of neg and emits partial denominators
D[i] = sum_{k in slice} exp(neg[i,k]); the host sums the 4 partials per
row-group, takes log, and does the final tiny reduction in float64.

Host-side prep is layout/cast only: fp8e4 casts and a pre-transposed copy
of the batch1 strip (b1T) so the device does zero b1-side transposes.  All
normalization math stays on device:
  - ssq/rsqrt of both batches on device (DVE/GpSimd STT + ACT Ln/Exp)
  - batch2 rows are normalized (x10 = 1/TEMP) during the PE diag-transpose
  - batch1 rows are normalized by folding inv1[i] into the ACT Exp *scale
    vector* (per-partition AP) -- the main matmul consumes raw fp8 b1T.
Main matmul runs fp8 DoubleRow (2 c-chunks per pass).  A single manual
ACT table load (natural_log_exp_and_others serves Exp/Ln/Copy/Square)
avoids the per-switch 1.28us table reloads.
"""

import numpy as np
import ml_dtypes

B = 4096
C = 512
NCORES = 8
MR = 2                    # row groups (batch1 split)
MC = 4                    # col groups (batch2 split)
ROWS = B // MR            # 2048 batch1 rows per core
K = B // MC               # 1024 batch2 rows per core
MB = ROWS // 128          # 16 i-blocks
KB = K // 128             # 8 k-blocks
CC = C // 128             # 4 contraction chunks
TEMP = 0.1
N_TERMS = B * (B - 1) // 2
ACT_TABLE_LN_EXP = 6      # natural_log_exp_and_others in act_info.json

_CACHE = {}

CFG = {
    "warmup": 20,         # junk PE matmuls to ramp the clock before real work
    "off_ms": (4, 8, 12),  # i-blocks whose exp-rowsum runs on DVE (fast exp2
                           # bitcast trick) instead of ACT
}
# Schraudolph-style fast exp: exp(x) ~= bitcast_f32(int(A*x + B)); B tuned on
# the actual neg distribution for ~1e-4 mean bias on the row sums.
FEXP_A = 12102203.161561485          # 2^23 / ln(2)
FEXP_B = float(127 * 2**23 - 482000)
RSQRT_MAGIC = 0x5F3759DF


def build_bass():
    import concourse.bass as bass
    import concourse.bacc as bacc
    import concourse.tile as tile
    from concourse import mybir
    from contextlib import ExitStack

    fp32 = mybir.dt.float32
    fp8 = mybir.dt.float8e4
    i32 = mybir.dt.int32
    AF = mybir.ActivationFunctionType
    ALU = mybir.AluOpType
    PM = mybir.MatmulPerfMode
    AX = mybir.AxisListType

    nc = bacc.Bacc("TRN2", target_bir_lowering=False, debug=False,
                   num_devices=NCORES)

    b1t = nc.dram_tensor("b1t", [C, ROWS], fp8, kind="ExternalInput")
    b1n_d = nc.dram_tensor("b1n", [ROWS, C], fp8, kind="ExternalInput")
    b2n_d = nc.dram_tensor("b2n", [K, C], fp8, kind="ExternalInput")
    ident = nc.dram_tensor("ident", [128, 128], fp8, kind="ExternalInput")
    out = nc.dram_tensor("out", [128, MB + CC], fp32, kind="ExternalOutput")

    with tile.TileContext(nc) as tc, ExitStack() as ctx:
        sb = ctx.enter_context(tc.tile_pool(name="sb", bufs=1))
        dumps = ctx.enter_context(tc.tile_pool(name="dumps", bufs=3))
        pt = ctx.enter_context(tc.tile_pool(name="pt", bufs=2, space="PSUM"))
        pneg = ctx.enter_context(tc.tile_pool(name="pneg", bufs=3, space="PSUM"))

        b1T = sb.tile([128, CC, ROWS], fp8, name="b1T")
        b1n = sb.tile([128, MB, C], fp8, name="b1n")
        b2n = sb.tile([128, KB, C], fp8, name="b2n")
        identb = sb.tile([128, 128], fp8, name="identb")
        b2sT = sb.tile([128, CC, K], fp8, name="b2sT")
        diag2 = sb.tile([128, KB, 128], fp8, name="diag2")
        ssq1 = sb.tile([128, MB], fp32, name="ssq1")
        ssq2 = sb.tile([128, KB], fp32, name="ssq2")
        ln1 = sb.tile([128, MB], fp32, name="ln1")
        ln2 = sb.tile([128, KB], fp32, name="ln2")
        invn1 = sb.tile([128, MB], fp32, name="invn1")
        invn1f8 = sb.tile([128, MB], fp8, name="invn1f8")
        invn2s = sb.tile([128, KB], fp32, name="invn2s")
        stage = sb.tile([128, MB + CC], fp32, name="stage")
        noff = len(CFG["off_ms"])
        ssqoff = sb.tile([128, max(noff, 1)], fp32, name="ssqoff")
        t0i = sb.tile([128, max(noff, 1)], i32, name="t0i")
        t1i = sb.tile([128, max(noff, 1)], i32, name="t1i")
        nh = sb.tile([128, max(noff, 1)], fp32, name="nh")
        ny1 = sb.tile([128, max(noff, 1)], fp32, name="ny1")
        ny2 = sb.tile([128, max(noff, 1)], fp32, name="ny2")
        avec = sb.tile([128, max(noff, 1)], fp32, name="avec")

        # single ACT table that serves Exp/Ln/Copy/Square for the whole kernel
        nc.scalar.add_instruction(mybir.InstLoadActFuncSet(
            name=nc.get_next_instruction_name(), ins=[], outs=[],
            act_func_set_id=ACT_TABLE_LN_EXP))

        # ---- input DMAs (gpsimd SWDGE: lowest issue latency; b2n first, in
        # two chunks so ssq2 of the first k-blocks starts sooner) -------------
        b2ap = b2n_d.ap().rearrange("(kb p) c -> p kb c", p=128)
        nc.gpsimd.dma_start(b2n[:, 0:4, :], b2ap[:, 0:4, :])
        nc.gpsimd.dma_start(b2n[:, 4:KB, :], b2ap[:, 4:KB, :])
        nc.gpsimd.dma_start(
            b1n[:, :, :], b1n_d.ap().rearrange("(mb p) c -> p mb c", p=128))
        nc.gpsimd.dma_start(
            b1T[:, :, :], b1t.ap().rearrange("(cc p) i -> p cc i", p=128))
        nc.sync.dma_start(identb[:, :], ident.ap())

        # ---- PE warmup: junk matmuls ramp the tensor-engine clock while the
        # DMAs land, so the transposes and main matmuls run at full p-state
        if CFG["warmup"]:
            warm = pt.tile([128, 128], fp32, name="warm", tag="pt")
            for _ in range(CFG["warmup"]):
                nc.tensor.matmul(warm[:, :], lhsT=identb[:, :],
                                 rhs=identb[:, :], start=True, stop=True)

        # ---- batch2 path (streamed in 2 groups of 4 k-blocks): --------------
        # ssq (split DVE/ACT) -> rsqrt(x10) -> diag (ACT) -> PE transpose
        # -> evac (DVE cast fp8)
        def ssq2_block(kb):
            dmp = dumps.tile([128, C], fp8, name="dssq2", tag="dssq2")
            nc.vector.scalar_tensor_tensor(
                out=dmp[:, :], in0=b2n[:, kb, :], scalar=1.0,
                in1=b2n[:, kb, :], op0=ALU.mult, op1=ALU.mult,
                accum_out=ssq2[:, kb:kb + 1])

        def transpose_block(kb, evac_eng):
            ptile = pt.tile([128, CC, 128], fp32, name="ptile", tag="pt")
            for cc in range(CC):
                nc.tensor.matmul(
                    ptile[:, cc, :],
                    lhsT=b2n[:, kb, cc * 128:(cc + 1) * 128],
                    rhs=diag2[:, kb, :],
                    start=True, stop=True)
            # evac split across DVE and ACT so the two groups drain in parallel
            if evac_eng == "act":
                nc.scalar.copy(
                    b2sT[:, :, kb * 128:(kb + 1) * 128], ptile[:, :, :])
            else:
                nc.vector.tensor_copy(
                    b2sT[:, :, kb * 128:(kb + 1) * 128], ptile[:, :, :])

        def ssq1_block(mb):
            dmp = dumps.tile([128, C], fp8, name="dssq1", tag="dssq1")
            nc.vector.scalar_tensor_tensor(
                out=dmp[:, :], in0=b1n[:, mb, :], scalar=1.0,
                in1=b1n[:, mb, :], op0=ALU.mult, op1=ALU.mult,
                accum_out=ssq1[:, mb:mb + 1])

        # all ssq2 first (dense DVE stream, overlapping the DMA chunks), with
        # the rsqrt pairs (ACT) slotted per granule; then both diag groups,
        # then the transposes with evacs split ACT (g0) / DVE (g1)
        for g in range(2):
            gs = slice(g * 4, (g + 1) * 4)
            for kb in range(g * 4, (g + 1) * 4):
                ssq2_block(kb)
            # 10/sqrt(x) == exp(-0.5 * ln(0.01 * x)); 10 = 1/TEMP
            nc.scalar.activation(ln2[:, gs], ssq2[:, gs], AF.Ln, scale=0.01)
            nc.scalar.activation(invn2s[:, gs], ln2[:, gs], AF.Exp, scale=-0.5)
        for kb in range(KB):
            nc.vector.tensor_scalar_mul(
                diag2[:, kb, :], identb[:, :], invn2s[:, kb:kb + 1])
        for kb in range(KB):
            transpose_block(kb, "act")

        # ---- batch1 stats: ssq on DVE (after the b2-path evacs) -------------
        for mb in range(MB):
            ssq1_block(mb)

        def rsqrt1_granule(g):
            gs = slice(g * 4, (g + 1) * 4)
            nc.scalar.activation(ln1[:, gs], ssq1[:, gs], AF.Ln)
            nc.scalar.activation(invn1[:, gs], ln1[:, gs], AF.Exp, scale=-0.5)

        # ---- DVE-side scale vector for the offloaded i-blocks:
        # avec = FEXP_A / sqrt(ssq1[off]) via bit-trick seed + 2 Newton steps
        off_ms = tuple(CFG["off_ms"])
        if off_ms:
            for j, mo in enumerate(off_ms):
                nc.vector.tensor_copy(ssqoff[:, j:j + 1], ssq1[:, mo:mo + 1])
            nos = slice(0, noff)
            nc.vector.tensor_scalar(
                t0i[:, nos], ssqoff[:, nos].bitcast(i32), 1, None,
                op0=ALU.logical_shift_right)
            nc.vector.tensor_scalar(
                t1i[:, nos], t0i[:, nos], -1, RSQRT_MAGIC,
                op0=ALU.mult, op1=ALU.add)
            y = t1i[:, nos].bitcast(fp32)
            for yn in (ny1, ny2):
                nc.vector.tensor_tensor(nh[:, nos], y, y, op=ALU.mult)
                nc.vector.tensor_tensor(
                    nh[:, nos], nh[:, nos], ssqoff[:, nos], op=ALU.mult)
                nc.vector.tensor_scalar(
                    nh[:, nos], nh[:, nos], -0.5, 1.5, op0=ALU.mult,
                    op1=ALU.add)
                nc.vector.tensor_tensor(yn[:, nos], y, nh[:, nos], op=ALU.mult)
                y = yn[:, nos]
            nc.vector.tensor_scalar_mul(avec[:, nos], y, FEXP_A)

        # ---- main: neg strip matmul (fp8 DoubleRow) + fused exp-rowsum ------
        # rsqrt granules interleave with the exp stream (ACT executes
        # in-order): granule g lands right before exp of i-block 4g.  The
        # off_ms i-blocks run the exp-rowsum on DVE (fast exp2 bitcast).
        for m in range(MB):
            if m % 4 == 0:
                rsqrt1_granule(m // 4)
            ntile = pneg.tile([128, 2, 512], fp32, name="ntile", tag="pneg")
            for kg in range(2):
                for mg in range(2):
                    nc.tensor.matmul(
                        ntile[:, mg, :],
                        lhsT=b1T[:, 2 * kg:2 * kg + 2, m * 128:(m + 1) * 128],
                        rhs=b2sT[:, 2 * kg:2 * kg + 2, mg * 512:(mg + 1) * 512],
                        start=(kg == 0), stop=(kg == 1),
                        perf_mode=PM.DoubleRow)
            nfull = ntile[:, :, :].rearrange("p a b -> p (a b)")
            if m in off_ms:
                j = off_ms.index(m)
                ti = dumps.tile([128, 1024], i32, name="dti", tag="dti")
                nc.vector.tensor_scalar(
                    ti[:, :], nfull, avec[:, j:j + 1], FEXP_B,
                    op0=ALU.mult, op1=ALU.add)
                nc.vector.tensor_reduce(
                    stage[:, m:m + 1], ti[:, :].bitcast(fp32),
                    axis=AX.X, op=ALU.add)
            else:
                dmp = dumps.tile([128, 1024], fp8, name="dexp", tag="dexp")
                nc.scalar.activation(
                    dmp[:, :], nfull,
                    AF.Exp, scale=invn1[:, m:m + 1],
                    accum_out=stage[:, m:m + 1])

        # ---- s partial: s[c] = sum_i b1[i,c] * inv1[i] over this strip ------
        nc.vector.tensor_copy(invn1f8[:, :], invn1[:, :])
        psum_s = pt.tile([128, CC], fp32, name="psum_s", tag="pt")
        for cc in range(CC):
            for mb in range(MB):
                nc.tensor.matmul(
                    psum_s[:, cc:cc + 1],
                    lhsT=b1n[:, mb, cc * 128:(cc + 1) * 128],
                    rhs=invn1f8[:, mb:mb + 1],
                    start=(mb == 0), stop=(mb == MB - 1))
        nc.vector.tensor_copy(stage[:, MB:MB + CC], psum_s[:, :])

        nc.sync.dma_start(out.ap(), stage[:, :])

    nc.compile()
    return nc


def _get_nc():
    key = ("nc", tuple(sorted(CFG.items())))
    if key not in _CACHE:
        _CACHE[key] = build_bass()
    return _CACHE[key]


def make_in_maps(batch1, batch2):
    f8 = ml_dtypes.float8_e4m3
    batch1 = np.ascontiguousarray(np.asarray(batch1, dtype=np.float32))
    batch2 = np.ascontiguousarray(np.asarray(batch2, dtype=np.float32))
    eye = np.eye(128, dtype=f8)
    maps = []
    b1s = []
    for r in range(MR):
        strip = batch1[r * ROWS:(r + 1) * ROWS]
        b1s.append({
            "b1t": np.ascontiguousarray(strip.T.astype(f8)),
            "b1n": np.ascontiguousarray(strip.astype(f8)),
        })
    b2s = [np.ascontiguousarray(batch2[c * K:(c + 1) * K].astype(f8))
           for c in range(MC)]
    for core in range(NCORES):
        r, c = divmod(core, MC)
        maps.append({
            "b1t": b1s[r]["b1t"], "b1n": b1s[r]["b1n"],
            "b2n": b2s[c], "ident": eye,
        })
    return maps


def combine(results):
    """Host-side gather.  results[core]["out"] is [128, MB+CC] fp32:
    cols 0..MB-1 = D partials (row i = m*128 + p of the core's strip),
    cols MB..    = s partial [c split over (cc, p)]."""
    # ld: sum the 4 col-group partials per row-group, then log
    ld = np.empty(B, dtype=np.float64)
    for r in range(MR):
        d = np.zeros((128, MB), dtype=np.float64)
        for c in range(MC):
            d += np.asarray(results[r * MC + c]["out"][:, :MB], np.float64)
        # row index within strip = m*128 + p  ->  [MB, 128] transposed flat
        ld[r * ROWS:(r + 1) * ROWS] = np.log(d.T.reshape(-1))
    # s: each row-group leader computed the full strip partial; sum groups
    s = np.zeros(C, dtype=np.float64)
    for r in range(MR):
        sp = np.asarray(results[r * MC]["out"][:, MB:MB + CC], np.float64)
        s += sp.T.reshape(-1)  # c = cc*128 + p
    term1 = np.dot(np.arange(B, dtype=np.float64), ld)
    tri = (np.dot(s, s) / TEMP - B / TEMP) / 2.0
    return np.asarray((term1 - tri) / N_TERMS, dtype=np.float32)


def run_hw(in_maps, trace=False, **kwargs):
    from concourse.bass_utils import run_bass_kernel_spmd
    return run_bass_kernel_spmd(_get_nc(), in_maps,
                                core_ids=list(range(NCORES)),
                                trace=trace, **kwargs)


def kernel(batch1, batch2):
    res = run_hw(make_in_maps(batch1, batch2))
    return combine(res.results)
